# revision 34
# baseline (speedup 1.0000x reference)
"""Causal self-attention (B=2, T=2048, E=1024, H=16) on 8 trn2 NeuronCores.

Sharding: core = b*4 + g  (b = batch index, g = head-group of 4 heads).
Each core computes its 4 heads' attention for its batch plus a partial
output projection; the host sums the 4 partials per batch.

RoPE: since sin/cos repeat across the two half-blocks (s[d+32]=s[d]),
rot(q*s) = rot(q)*s, so the kernel multiplies the projected q by sin
first (which doubles as the PSUM->SBUF staging copy) and applies
rotate_half as a single 128x128 signed-permutation matmul; q' is then
q*cos + rot(q*s) -- no duplicated rotated-weight projections.
Layout: qT tile tau rows [64l : 64l+64) = head 2tau+l dims 0..63, T on
free.  Scores are computed transposed (S^T = K Q^T, k on partitions,
K=64 contraction) so softmax exp feeds the attention matmul with no
transposes; the two heads of a tile sit on disjoint PE row-groups and
psum banks so their score matmuls run concurrently.  V carries an
appended ones column so row 64 of the attention psum is the softmax
denominator; the reciprocal is taken straight from PSUM and the divide
is fused into the PSUM->SBUF evacuation of the attention rows.  All
matmul operands bf16; fp32 accumulation; exp skips fully-invalid
diagonal columns; diagonal score matmuls and AV matmuls are trimmed to
the causally valid q-range.  A short burst of throwaway matmuls on a
memset tile runs during the weight DMA prologue to hold the PE HAM
clock-gate at full rate.
"""

import numpy as np
import ml_dtypes

BF16 = ml_dtypes.bfloat16

B, T, E = 2, 2048, 1024
H, HD = 16, 64
G = 4             # head groups (cores per batch)
HL = H // G       # heads per core
DL = HL * HD      # local qkv dim = 256
TC = 512          # T chunk (matmul moving free dim)
NJ = T // TC      # 4 q-windows
KC = 128          # k-chunk (contraction tile for attention)
NC_ = T // KC     # 16 k-chunks
SCALE = 1.0 / float(np.sqrt(HD))
WARM_MM = 26      # PE warm-up matmuls issued during the DMA prologue

_CACHE = {}


def _build_bass():
    import concourse.mybir as mybir
    import concourse.tile as tile
    from concourse import bacc

    f32 = mybir.dt.float32
    bf16 = mybir.dt.bfloat16
    EXP = mybir.ActivationFunctionType.Exp

    nc = bacc.Bacc("TRN2", target_bir_lowering=False, debug=False)
    xt_d = nc.dram_tensor("xt", [E, T], bf16, kind="ExternalInput").ap()
    w_d = nc.dram_tensor("w", [E, 3 * DL], bf16, kind="ExternalInput").ap()
    wo_d = nc.dram_tensor("wo", [DL, E], bf16, kind="ExternalInput").ap()
    cos_d = nc.dram_tensor("cosf", [128, T], bf16, kind="ExternalInput").ap()
    sin_d = nc.dram_tensor("sinf", [128, T], bf16, kind="ExternalInput").ap()
    r2_d = nc.dram_tensor("r2", [128, 128], bf16, kind="ExternalInput").ap()
    y_d = nc.dram_tensor("y", [T, E], bf16, kind="ExternalOutput").ap()

    NKK = E // KC  # 8 contraction chunks for the projections

    with tile.TileContext(nc) as tc:
        with (
            tc.tile_pool(name="consts", bufs=1) as consts,
            tc.tile_pool(name="stp", bufs=2, space="PSUM") as stp,
            tc.tile_pool(name="avp", bufs=1, space="PSUM") as avp,
            tc.tile_pool(name="tmp_sb", bufs=4) as tmp_sb,
            tc.tile_pool(name="est_sb", bufs=16) as est_sb,
            tc.tile_pool(name="attn_sb", bufs=1) as attn_sb,
            tc.tile_pool(name="ysb_p", bufs=2) as ysb_p,
            tc.tile_pool(name="small_sb", bufs=2) as small_sb,
        ):
            # ---- PE warm-up: keep the HAM clock-gate open while the
            # weight/activation DMAs land (results never read) ----
            wu = consts.tile([128, TC], bf16, tag="wu")
            nc.gpsimd.memset(wu, 0.03125)
            ones128 = consts.tile([128, 64], bf16, tag="ones128")
            nc.gpsimd.memset(ones128, 1.0)
            wups = stp.tile([128, 2 * TC], f32, tag="st", name="wup")
            for i in range(WARM_MM):
                nc.tensor.matmul(wups[:, 0:TC], lhsT=wu[:, 0:128], rhs=wu,
                                 start=True, stop=True)

            # ---- constants.  DMA order = first-window critical path;
            # the small consts ride the scalar queue so they don't delay
            # the weight/activation stream on the sync queue ----
            xt, w = [], []
            for i in range(NKK):
                eng = nc.sync if i % 2 == 0 else nc.scalar
                tw = consts.tile([KC, 3 * DL], bf16, tag=f"w{i}", name=f"w{i}")
                eng.dma_start(out=tw, in_=w_d[i * KC:(i + 1) * KC, :])
                w.append(tw)
                t = consts.tile([KC, T], bf16, tag=f"xt{i}", name=f"xt{i}")
                eng.dma_start(
                    out=t[:, 0:TC], in_=xt_d[i * KC:(i + 1) * KC, 0:TC])
                xt.append(t)
                if i == 0:
                    r2 = consts.tile([128, 128], bf16, tag="r2")
                    nc.scalar.dma_start(out=r2, in_=r2_d)
                if i == 1:
                    cosf = consts.tile([128, T], bf16, tag="cosf")
                    nc.sync.dma_start(out=cosf, in_=cos_d)
                    sinf = consts.tile([128, T], bf16, tag="sinf")
                    nc.sync.dma_start(out=sinf, in_=sin_d)
            wo = []
            for tau in range(2):
                t = consts.tile([128, E], bf16, tag=f"wo{tau}", name=f"wo{tau}")
                nc.scalar.dma_start(
                    out=t, in_=wo_d[tau * 128:(tau + 1) * 128, :])
                wo.append(t)
            for jj in range(1, NJ):
                for i in range(NKK):
                    eng = nc.sync if (jj > 1 or i % 2 == 0) else nc.scalar
                    eng.dma_start(
                        out=xt[i][:, jj * TC:(jj + 1) * TC],
                        in_=xt_d[i * KC:(i + 1) * KC, jj * TC:(jj + 1) * TC])

            # resident projection outputs (natural head-contiguous layout)
            qn = [[consts.tile([128, TC], bf16, tag=f"qn{tau}_{j}",
                               name=f"qn{tau}_{j}") for j in range(NJ)]
                  for tau in range(2)]
            kn = [[consts.tile([128, TC], bf16, tag=f"kn{tau}_{j}",
                               name=f"kn{tau}_{j}") for j in range(NJ)]
                  for tau in range(2)]
            # v tiles padded to 128 dims/head (64 v dims | ones | zeros) so
            # the AV weight loads are full 128-column FWL loads
            vsb = [consts.tile([128, HL * 128], bf16, tag=f"v{c}",
                               name=f"v{c}") for c in range(NC_)]

            def emit_proj_qk_mm(jj, base, tau, dst):
                """Projection matmuls + u = q*sin for head pair tau of
                window jj; the rot-matmul + combine are emitted later by
                emit_proj_fin so intervening PE work hides the DVE wait."""
                js = slice(jj * TC, (jj + 1) * TC)
                ps = stp.tile([128, 2 * TC], f32, tag="st",
                              name=f"pqk{base}_{tau}_{jj}")
                cc = base + 128 * tau
                for kk in range(NKK):
                    nc.tensor.matmul(
                        ps[:, 0:TC],
                        lhsT=w[kk][:, cc:cc + 128],
                        rhs=xt[kk][:, js],
                        start=(kk == 0), stop=(kk == NKK - 1))
                u = tmp_sb.tile([128, TC], bf16, tag="ropeU",
                                name=f"ru{base}_{tau}_{jj}")
                nc.vector.tensor_mul(u, ps[:, 0:TC], sinf[:, js])
                return (ps, u, jj, tau, dst)

            def emit_proj_fin(ctx):
                ps, u, jj, tau, dst = ctx
                js = slice(jj * TC, (jj + 1) * TC)
                nc.tensor.matmul(ps[:, TC:2 * TC], lhsT=r2, rhs=u,
                                 start=True, stop=True)
                qk = 0 if dst is qn else 1
                ta = tmp_sb.tile([128, TC], f32, tag="ropeA",
                                 name=f"ra{qk}_{tau}_{jj}")
                nc.vector.tensor_mul(ta, ps[:, 0:TC], cosf[:, js])
                nc.vector.tensor_add(dst[tau][jj], ta, ps[:, TC:2 * TC])

            def emit_proj_v(jj):
                js0 = jj * (TC // KC)
                ps = stp.tile([128, 4 * DL], f32, tag="st", name=f"pv{jj}")
                for tt in range(TC // KC):
                    c = js0 + tt
                    for kk in range(NKK):
                        nc.tensor.matmul(
                            ps[:, tt * DL:(tt + 1) * DL],
                            lhsT=xt[kk][:, c * KC:(c + 1) * KC],
                            rhs=w[kk][:, 2 * DL:3 * DL],
                            start=(kk == 0), stop=(kk == NKK - 1))
                for tt in range(TC // KC):
                    c = js0 + tt
                    vv = vsb[c].rearrange("p (h d) -> p h d", h=HL)
                    nc.gpsimd.memset(vv[:, :, 64:128], 0.0)
                    nc.gpsimd.memset(vv[:, :, 64:65], 1.0)
                    nc.vector.tensor_copy(
                        vv[:, :, 0:64],
                        ps[:, tt * DL:(tt + 1) * DL].rearrange(
                            "p (h d) -> p h d", h=HL))

            def emit_proj_mm(jj, part):
                if part == 0:
                    return emit_proj_qk_mm(jj, 0, 0, qn)
                elif part == 1:
                    return emit_proj_qk_mm(jj, 0, 1, qn)
                elif part == 2:
                    return emit_proj_qk_mm(jj, DL, 0, kn)
                elif part == 3:
                    return emit_proj_qk_mm(jj, DL, 1, kn)
                emit_proj_v(jj)
                return None

            def emit_y(jj, part, copy_eng=None):
                n, th = divmod(part, 2)
                at = ats[jj]
                yp = stp.tile([128, 2 * TC], f32, tag="st",
                              name=f"yp{jj}_{n}_{th}")
                for ti in range(2):
                    tt = 2 * th + ti
                    for tau in range(2):
                        nc.tensor.matmul(
                            yp[:, ti * TC:(ti + 1) * TC],
                            lhsT=at[tau][:, tt * KC:(tt + 1) * KC],
                            rhs=wo[tau][:, n * TC:(n + 1) * TC],
                            start=(tau == 0), stop=(tau == 1))
                ys = ysb_p.tile([128, 2 * TC], bf16, tag="y",
                                name=f"ys{jj}_{n}_{th}")
                if copy_eng is nc.scalar:
                    nc.scalar.copy(ys, yp)
                else:
                    nc.vector.tensor_copy(ys, yp)
                nc.sync.dma_start(
                    out=y_d[jj * TC + th * 2 * KC:
                            jj * TC + (th + 1) * 2 * KC,
                            n * TC:(n + 1) * TC].rearrange(
                                "(a p) c -> p a c", p=128),
                    in_=ys.rearrange("p (a c) -> p a c", a=2))

            # prologue: projections for window 0, software-pipelined so each
            # rot-matmul waits behind the next projection's matmuls
            pend = []
            for part in range(5):
                ctx = emit_proj_mm(0, part)
                if len(pend) > 0:
                    emit_proj_fin(pend.pop(0))
                if ctx is not None:
                    pend.append(ctx)
            while pend:
                emit_proj_fin(pend.pop(0))
            ats = []
            pre_ests = {}              # (j, c) -> [est01, est23] prefetched

            def emit_scores_est(j, c):
                """Score pair + exp for chunk c of window j; returns the two
                est tiles.  Per-tau emission so the two score psum slots
                double-buffer across half-chunks."""
                nch = 4 * (j + 1)
                d = c - 4 * j
                qoff = KC * d if d > 0 else 0
                ests = []
                for tau in range(2):
                    stt = stp.tile([128, 2 * TC], f32, tag="st",
                                   name=f"st{j}_{c}_{tau}")
                    for ll in range(2):
                        w0c = ll * TC
                        nc.tensor.matmul(
                            stt[:, w0c + qoff:w0c + TC],
                            lhsT=kn[tau][c // 4][
                                64 * ll:64 * ll + 64,
                                (c % 4) * KC:(c % 4 + 1) * KC],
                            rhs=qn[tau][j][64 * ll:64 * ll + 64, qoff:],
                            start=True, stop=True,
                            tile_position=(64 * ll, 0))
                    est = est_sb.tile([128, 2 * TC], bf16, tag="est",
                                      name=f"est{j}_{c}_{tau}")
                    if qoff:
                        nc.scalar.activation(
                            est.rearrange("p (w c) -> p w c", w=2)[
                                :, :, qoff:],
                            stt.rearrange("p (w c) -> p w c", w=2)[
                                :, :, qoff:],
                            EXP, scale=SCALE)
                    else:
                        nc.scalar.activation(est, stt, EXP, scale=SCALE)
                    if d >= 0:
                        # zero the upper-triangular (future) part of the
                        # diagonal KC-block, on the (otherwise idle) gpsimd
                        for ll in range(2):
                            bs = slice(ll * TC + KC * d,
                                       ll * TC + KC * (d + 1))
                            nc.gpsimd.affine_select(
                                out=est[:, bs], in_=est[:, bs],
                                compare_op=mybir.AluOpType.is_ge, fill=0.0,
                                base=0, pattern=[[1, KC]],
                                channel_multiplier=-1)
                    ests.append(est)
                return ests

            def emit_av(j, c, ests, av4):
                nch = 4 * (j + 1)
                d = c - 4 * j
                coff = KC * d if d > 0 else 0
                for h in range(HL):
                    nc.tensor.matmul(
                        av4[0:128, h * TC + coff:(h + 1) * TC],
                        lhsT=vsb[c][:, 128 * h:128 * h + 128],
                        rhs=ests[h // 2][:, (h % 2) * TC + coff:
                                         (h % 2 + 1) * TC],
                        start=(c == 0), stop=(c == nch - 1))

            # exp of this many leading chunks of window j is precomputed
            # during window j-1 (fills ACT while that window is PE-bound)
            PREFETCH = [0, 0, 0, 0]

            for j in range(NJ):
                nch = 4 * (j + 1)          # causal k-chunks for this window
                # interleave next window's projection emission and the
                # previous window's output projection into the chunk loop
                proj_at = {}
                if j + 1 < NJ:
                    for part in range(5):
                        cpos = max(0, (nch * (part + 1)) // 6 - 1)
                        proj_at.setdefault(cpos, []).append(("p", part))
                if j > 0:
                    for part in range(4):
                        cpos = max(0, (nch * (part + 1)) // 5 - 2)
                        proj_at.setdefault(cpos, []).append(("y", part))
                if j + 1 < NJ:
                    for k in range(PREFETCH[j + 1]):
                        cpos = max(0, (nch * (k + 3)) // (PREFETCH[j + 1] + 3)
                                   - 1)
                        proj_at.setdefault(cpos, []).append(("e", k))

                at = [attn_sb.tile([128, TC], bf16, tag=f"attn{tau}_{j}",
                                   name=f"attn{tau}_{j}") for tau in range(2)]
                ats.append(at)
                av4 = avp.tile([128, 4 * TC], f32, tag="av", name=f"av_{j}")
                nxt = pre_ests.pop((j, 0), None)
                if nxt is None:
                    nxt = emit_scores_est(j, 0)
                for c in range(nch):
                    ests = nxt
                    # emit the next chunk's scores first so the scalar
                    # engine's exp queue is never starved behind the
                    # projection matmul blocks
                    if c + 1 < nch:
                        nxt = pre_ests.pop((j, c + 1), None)
                        if nxt is None:
                            nxt = emit_scores_est(j, c + 1)
                    evs = proj_at.get(c, ())
                    # projection matmuls go before the AV matmuls (they fill
                    # the PE while ACT runs exp); the rot-matmul + rope
                    # combine go after, hiding the u-multiply latency
                    pend = []
                    for kind, part in evs:
                        if kind == "p":
                            ctx = emit_proj_mm(j + 1, part)
                            if ctx is not None:
                                pend.append(ctx)
                    for ctx in pend:
                        emit_proj_fin(ctx)
                    emit_av(j, c, ests, av4)
                    for kind, part in evs:
                        if kind == "y":
                            emit_y(j - 1, part)
                        elif kind == "e":
                            pre_ests[(j + 1, part)] = emit_scores_est(
                                j + 1, part)

                # softmax: one scalar-engine copy per head evacuates a whole
                # AV psum bank (raw rows 0:64 + denominator row 64), so the
                # next window's AV can restart in ~2us with no DVE or bank
                # cross-serialization.  The denominator is then broadcast
                # across partitions with K=1 matmuls (ones^T @ den, ~0.2us
                # each) and the divide runs off the critical path.
                ag = []
                for h in range(HL):
                    hw = slice(h * TC, (h + 1) * TC)
                    a = small_sb.tile([65, TC], bf16, tag=f"ag{h % 2}",
                                      name=f"ag{j}_{h}")
                    # heads 0/1 on DVE, heads 2/3 on ACT: the four copies
                    # touch four different psum banks, so the two engines
                    # drain the AV psum concurrently (~1us)
                    if h < 2:
                        nc.vector.tensor_copy(a, av4[0:65, hw])
                    else:
                        nc.scalar.copy(a, av4[0:65, hw])
                    ag.append(a)
                db = [stp.tile([64, 2 * TC], f32, tag="st", name=f"db{j}_{t}")
                      for t in range(2)]
                for tau in range(2):
                    for par in range(2):
                        nc.tensor.matmul(
                            db[tau][:, par * TC:(par + 1) * TC],
                            lhsT=ones128[64:65, :],
                            rhs=ag[2 * tau + par][64:65, :],
                            start=True, stop=True,
                            tile_position=(64, 0))
                rbs = []
                for tau in range(2):
                    rb = small_sb.tile([64, 2 * TC], f32, tag="rbcast",
                                       name=f"rb{j}_{tau}")
                    rbs.append(rb)
                last = j == NJ - 1
                # divide in column halves (reciprocals split the same way,
                # so the first half's divide starts as early as possible);
                # in the epilogue each half's output-projection parts are
                # interleaved right behind it
                for half in range(2):
                    cs = slice(half * (TC // 2), (half + 1) * (TC // 2))
                    for tau in range(2):
                        for par in range(2):
                            rcs = slice(par * TC + half * (TC // 2),
                                        par * TC + (half + 1) * (TC // 2))
                            nc.vector.reciprocal_approx_fast(
                                out=rbs[tau][:, rcs], in_=db[tau][:, rcs])
                    for tau in range(2):
                        for par in range(2):
                            nc.vector.tensor_mul(
                                at[tau][64 * par:64 * par + 64, cs],
                                ag[2 * tau + par][0:64, cs],
                                rbs[tau][:, par * TC + half * (TC // 2):
                                         par * TC + (half + 1) * (TC // 2)])
                    if last:
                        # parts with th == half read the freshly divided cols
                        emit_y(j, half, copy_eng=nc.scalar)
                        emit_y(j, 2 + half, copy_eng=None)

    nc.compile()
    return nc


def _host_inputs(x, cos, sin, w_qkv, w_out):
    """Shard + lay out the full inputs for the 8 cores."""
    # natural-layout tables: row 64*l + d = cos/sin[t, d]
    cosf = np.ascontiguousarray(np.tile(cos.T, (2, 1))).astype(BF16)
    sinf = np.ascontiguousarray(np.tile(sin.T, (2, 1))).astype(BF16)

    xts = [np.ascontiguousarray(x[b].T).astype(BF16) for b in range(B)]

    # rotate_half as a stationary matmul operand: out = r2.T @ u
    r2 = np.zeros((128, 128), dtype=np.float32)
    for b in (0, 64):
        for dd in range(32):
            r2[b + 32 + dd, b + dd] = -1.0
            r2[b + dd, b + 32 + dd] = 1.0
    r2 = r2.astype(BF16)

    in_maps = []
    for core in range(8):
        b, g = divmod(core, G)
        qblk = w_qkv[:, G * g * HD:(G * g + HL) * HD]
        kblk = w_qkv[:, E + G * g * HD:E + (G * g + HL) * HD]
        vblk = w_qkv[:, 2 * E + DL * g:2 * E + DL * (g + 1)]
        wl = np.concatenate([qblk, kblk, vblk], axis=1).astype(BF16)  # (E, 768)
        wol = np.ascontiguousarray(w_out[DL * g:DL * (g + 1), :]).astype(BF16)
        in_maps.append({
            "xt": xts[b], "w": wl, "wo": wol, "cosf": cosf, "sinf": sinf,
            "r2": r2,
        })
    return in_maps


def kernel(x, cos, sin, w_qkv, w_out):
    from concourse import bass_utils

    if "nc" not in _CACHE:
        _CACHE["nc"] = _build_bass()
    nc = _CACHE["nc"]

    in_maps = _host_inputs(
        np.asarray(x, dtype=np.float32), np.asarray(cos, dtype=np.float32),
        np.asarray(sin, dtype=np.float32), np.asarray(w_qkv, dtype=np.float32),
        np.asarray(w_out, dtype=np.float32))

    res = bass_utils.run_bass_kernel_spmd(nc, in_maps, core_ids=list(range(8)))

    y = np.zeros((B, T, E), dtype=np.float32)
    for core in range(8):
        b = core // G
        y[b] += res.results[core]["y"].astype(np.float32)
    return y


# revision 35
# speedup vs baseline: 1.0137x; 1.0137x over previous
"""Causal self-attention (B=2, T=2048, E=1024, H=16) on 8 trn2 NeuronCores.

Sharding: core = b*4 + g  (b = batch index, g = head-group of 4 heads).
Each core computes its 4 heads' attention for its batch plus a partial
output projection; the host sums the 4 partials per batch.

RoPE: since sin/cos repeat across the two half-blocks (s[d+32]=s[d]),
rot(q*s) = rot(q)*s, so the kernel multiplies the projected q by sin
first (which doubles as the PSUM->SBUF staging copy) and applies
rotate_half as a single 128x128 signed-permutation matmul; q' is then
q*cos + rot(q*s) -- no duplicated rotated-weight projections.
Layout: qT tile tau rows [64l : 64l+64) = head 2tau+l dims 0..63, T on
free.  Scores are computed transposed (S^T = K Q^T, k on partitions,
K=64 contraction) so softmax exp feeds the attention matmul with no
transposes; the two heads of a tile sit on disjoint PE row-groups and
psum banks so their score matmuls run concurrently.  V carries an
appended ones column so row 64 of the attention psum is the softmax
denominator; the reciprocal is taken straight from PSUM and the divide
is fused into the PSUM->SBUF evacuation of the attention rows.  All
matmul operands bf16; fp32 accumulation; exp skips fully-invalid
diagonal columns; diagonal score matmuls and AV matmuls are trimmed to
the causally valid q-range.  A short burst of throwaway matmuls on a
memset tile runs during the weight DMA prologue to hold the PE HAM
clock-gate at full rate.
"""

import numpy as np
import ml_dtypes

BF16 = ml_dtypes.bfloat16

B, T, E = 2, 2048, 1024
H, HD = 16, 64
G = 4             # head groups (cores per batch)
HL = H // G       # heads per core
DL = HL * HD      # local qkv dim = 256
TC = 512          # T chunk (matmul moving free dim)
NJ = T // TC      # 4 q-windows
KC = 128          # k-chunk (contraction tile for attention)
NC_ = T // KC     # 16 k-chunks
SCALE = 1.0 / float(np.sqrt(HD))
WARM_MM = 26      # PE warm-up matmuls issued during the DMA prologue

_CACHE = {}


def _build_bass():
    import concourse.mybir as mybir
    import concourse.tile as tile
    from concourse import bacc

    f32 = mybir.dt.float32
    bf16 = mybir.dt.bfloat16
    EXP = mybir.ActivationFunctionType.Exp

    nc = bacc.Bacc("TRN2", target_bir_lowering=False, debug=False)
    xt_d = nc.dram_tensor("xt", [E, T], bf16, kind="ExternalInput").ap()
    w_d = nc.dram_tensor("w", [E, 3 * DL], bf16, kind="ExternalInput").ap()
    wo_d = nc.dram_tensor("wo", [DL, E], bf16, kind="ExternalInput").ap()
    cos_d = nc.dram_tensor("cosf", [128, T], bf16, kind="ExternalInput").ap()
    sin_d = nc.dram_tensor("sinf", [128, T], bf16, kind="ExternalInput").ap()
    r2_d = nc.dram_tensor("r2", [128, 128], bf16, kind="ExternalInput").ap()
    y_d = nc.dram_tensor("y", [T, E], bf16, kind="ExternalOutput").ap()

    NKK = E // KC  # 8 contraction chunks for the projections

    with tile.TileContext(nc) as tc:
        with (
            tc.tile_pool(name="consts", bufs=1) as consts,
            tc.tile_pool(name="stp", bufs=2, space="PSUM") as stp,
            tc.tile_pool(name="avp", bufs=1, space="PSUM") as avp,
            tc.tile_pool(name="tmp_sb", bufs=4) as tmp_sb,
            tc.tile_pool(name="est_sb", bufs=16) as est_sb,
            tc.tile_pool(name="attn_sb", bufs=1) as attn_sb,
            tc.tile_pool(name="ysb_p", bufs=2) as ysb_p,
            tc.tile_pool(name="small_sb", bufs=2) as small_sb,
        ):
            # ---- PE warm-up: keep the HAM clock-gate open while the
            # weight/activation DMAs land (results never read) ----
            wu = consts.tile([128, TC], bf16, tag="wu")
            nc.gpsimd.memset(wu, 0.03125)
            ones128 = consts.tile([128, 64], bf16, tag="ones128")
            nc.gpsimd.memset(ones128, 1.0)
            wups = stp.tile([128, 2 * TC], f32, tag="st", name="wup")
            for i in range(WARM_MM):
                nc.tensor.matmul(wups[:, 0:TC], lhsT=wu[:, 0:128], rhs=wu,
                                 start=True, stop=True)

            # ---- constants.  DMA order = first-window critical path;
            # the small consts ride the scalar queue so they don't delay
            # the weight/activation stream on the sync queue ----
            xt, w = [], []
            for i in range(NKK):
                tw = consts.tile([KC, 3 * DL], bf16, tag=f"w{i}", name=f"w{i}")
                nc.sync.dma_start(out=tw, in_=w_d[i * KC:(i + 1) * KC, :])
                w.append(tw)
                t = consts.tile([KC, T], bf16, tag=f"xt{i}", name=f"xt{i}")
                nc.sync.dma_start(
                    out=t[:, 0:TC], in_=xt_d[i * KC:(i + 1) * KC, 0:TC])
                xt.append(t)
                if i == 0:
                    r2 = consts.tile([128, 128], bf16, tag="r2")
                    nc.scalar.dma_start(out=r2, in_=r2_d)
                if i == 1:
                    cosf = consts.tile([128, T], bf16, tag="cosf")
                    nc.scalar.dma_start(out=cosf, in_=cos_d)
                    sinf = consts.tile([128, T], bf16, tag="sinf")
                    nc.scalar.dma_start(out=sinf, in_=sin_d)
            wo = []
            for tau in range(2):
                t = consts.tile([128, E], bf16, tag=f"wo{tau}", name=f"wo{tau}")
                nc.scalar.dma_start(
                    out=t, in_=wo_d[tau * 128:(tau + 1) * 128, :])
                wo.append(t)
            for jj in range(1, NJ):
                for i in range(NKK):
                    nc.sync.dma_start(
                        out=xt[i][:, jj * TC:(jj + 1) * TC],
                        in_=xt_d[i * KC:(i + 1) * KC, jj * TC:(jj + 1) * TC])

            # resident projection outputs (natural head-contiguous layout)
            qn = [[consts.tile([128, TC], bf16, tag=f"qn{tau}_{j}",
                               name=f"qn{tau}_{j}") for j in range(NJ)]
                  for tau in range(2)]
            kn = [[consts.tile([128, TC], bf16, tag=f"kn{tau}_{j}",
                               name=f"kn{tau}_{j}") for j in range(NJ)]
                  for tau in range(2)]
            # v tiles padded to 128 dims/head (64 v dims | ones | zeros) so
            # the AV weight loads are full 128-column FWL loads
            vsb = [consts.tile([128, HL * 128], bf16, tag=f"v{c}",
                               name=f"v{c}") for c in range(NC_)]

            def emit_proj_qk_mm(jj, base, tau, dst):
                """Projection matmuls + u = q*sin for head pair tau of
                window jj; the rot-matmul + combine are emitted later by
                emit_proj_fin so intervening PE work hides the DVE wait."""
                js = slice(jj * TC, (jj + 1) * TC)
                ps = stp.tile([128, 2 * TC], f32, tag="st",
                              name=f"pqk{base}_{tau}_{jj}")
                cc = base + 128 * tau
                for kk in range(NKK):
                    nc.tensor.matmul(
                        ps[:, 0:TC],
                        lhsT=w[kk][:, cc:cc + 128],
                        rhs=xt[kk][:, js],
                        start=(kk == 0), stop=(kk == NKK - 1))
                u = tmp_sb.tile([128, TC], bf16, tag="ropeU",
                                name=f"ru{base}_{tau}_{jj}")
                nc.vector.tensor_mul(u, ps[:, 0:TC], sinf[:, js])
                return (ps, u, jj, tau, dst)

            def emit_proj_fin(ctx):
                ps, u, jj, tau, dst = ctx
                js = slice(jj * TC, (jj + 1) * TC)
                nc.tensor.matmul(ps[:, TC:2 * TC], lhsT=r2, rhs=u,
                                 start=True, stop=True)
                qk = 0 if dst is qn else 1
                ta = tmp_sb.tile([128, TC], f32, tag="ropeA",
                                 name=f"ra{qk}_{tau}_{jj}")
                nc.vector.tensor_mul(ta, ps[:, 0:TC], cosf[:, js])
                nc.vector.tensor_add(dst[tau][jj], ta, ps[:, TC:2 * TC])

            def emit_proj_v(jj):
                js0 = jj * (TC // KC)
                ps = stp.tile([128, 4 * DL], f32, tag="st", name=f"pv{jj}")
                for tt in range(TC // KC):
                    c = js0 + tt
                    for kk in range(NKK):
                        nc.tensor.matmul(
                            ps[:, tt * DL:(tt + 1) * DL],
                            lhsT=xt[kk][:, c * KC:(c + 1) * KC],
                            rhs=w[kk][:, 2 * DL:3 * DL],
                            start=(kk == 0), stop=(kk == NKK - 1))
                for tt in range(TC // KC):
                    c = js0 + tt
                    vv = vsb[c].rearrange("p (h d) -> p h d", h=HL)
                    nc.gpsimd.memset(vv[:, :, 64:128], 0.0)
                    nc.gpsimd.memset(vv[:, :, 64:65], 1.0)
                    nc.vector.tensor_copy(
                        vv[:, :, 0:64],
                        ps[:, tt * DL:(tt + 1) * DL].rearrange(
                            "p (h d) -> p h d", h=HL))

            def emit_proj_mm(jj, part):
                if part == 0:
                    return emit_proj_qk_mm(jj, 0, 0, qn)
                elif part == 1:
                    return emit_proj_qk_mm(jj, 0, 1, qn)
                elif part == 2:
                    return emit_proj_qk_mm(jj, DL, 0, kn)
                elif part == 3:
                    return emit_proj_qk_mm(jj, DL, 1, kn)
                emit_proj_v(jj)
                return None

            def emit_y(jj, part, copy_eng=None):
                n, th = divmod(part, 2)
                at = ats[jj]
                yp = stp.tile([128, 2 * TC], f32, tag="st",
                              name=f"yp{jj}_{n}_{th}")
                for ti in range(2):
                    tt = 2 * th + ti
                    for tau in range(2):
                        nc.tensor.matmul(
                            yp[:, ti * TC:(ti + 1) * TC],
                            lhsT=at[tau][:, tt * KC:(tt + 1) * KC],
                            rhs=wo[tau][:, n * TC:(n + 1) * TC],
                            start=(tau == 0), stop=(tau == 1))
                ys = ysb_p.tile([128, 2 * TC], bf16, tag="y",
                                name=f"ys{jj}_{n}_{th}")
                if copy_eng is nc.scalar:
                    nc.scalar.copy(ys, yp)
                else:
                    nc.vector.tensor_copy(ys, yp)
                nc.sync.dma_start(
                    out=y_d[jj * TC + th * 2 * KC:
                            jj * TC + (th + 1) * 2 * KC,
                            n * TC:(n + 1) * TC].rearrange(
                                "(a p) c -> p a c", p=128),
                    in_=ys.rearrange("p (a c) -> p a c", a=2))

            # prologue: projections for window 0, software-pipelined so each
            # rot-matmul waits behind the next projection's matmuls
            pend = []
            for part in range(5):
                ctx = emit_proj_mm(0, part)
                if len(pend) > 0:
                    emit_proj_fin(pend.pop(0))
                if ctx is not None:
                    pend.append(ctx)
            while pend:
                emit_proj_fin(pend.pop(0))
            ats = []
            pre_ests = {}              # (j, c) -> [est01, est23] prefetched

            def emit_scores_est(j, c):
                """Score pair + exp for chunk c of window j; returns the two
                est tiles.  Per-tau emission so the two score psum slots
                double-buffer across half-chunks."""
                nch = 4 * (j + 1)
                d = c - 4 * j
                qoff = KC * d if d > 0 else 0
                ests = []
                for tau in range(2):
                    stt = stp.tile([128, 2 * TC], f32, tag="st",
                                   name=f"st{j}_{c}_{tau}")
                    for ll in range(2):
                        w0c = ll * TC
                        nc.tensor.matmul(
                            stt[:, w0c + qoff:w0c + TC],
                            lhsT=kn[tau][c // 4][
                                64 * ll:64 * ll + 64,
                                (c % 4) * KC:(c % 4 + 1) * KC],
                            rhs=qn[tau][j][64 * ll:64 * ll + 64, qoff:],
                            start=True, stop=True,
                            tile_position=(64 * ll, 0))
                    est = est_sb.tile([128, 2 * TC], bf16, tag="est",
                                      name=f"est{j}_{c}_{tau}")
                    if qoff:
                        nc.scalar.activation(
                            est.rearrange("p (w c) -> p w c", w=2)[
                                :, :, qoff:],
                            stt.rearrange("p (w c) -> p w c", w=2)[
                                :, :, qoff:],
                            EXP, scale=SCALE)
                    else:
                        nc.scalar.activation(est, stt, EXP, scale=SCALE)
                    if d >= 0:
                        # zero the upper-triangular (future) part of the
                        # diagonal KC-block, on the (otherwise idle) gpsimd
                        for ll in range(2):
                            bs = slice(ll * TC + KC * d,
                                       ll * TC + KC * (d + 1))
                            nc.gpsimd.affine_select(
                                out=est[:, bs], in_=est[:, bs],
                                compare_op=mybir.AluOpType.is_ge, fill=0.0,
                                base=0, pattern=[[1, KC]],
                                channel_multiplier=-1)
                    ests.append(est)
                return ests

            def emit_av(j, c, ests, av4):
                nch = 4 * (j + 1)
                d = c - 4 * j
                coff = KC * d if d > 0 else 0
                for h in range(HL):
                    nc.tensor.matmul(
                        av4[0:128, h * TC + coff:(h + 1) * TC],
                        lhsT=vsb[c][:, 128 * h:128 * h + 128],
                        rhs=ests[h // 2][:, (h % 2) * TC + coff:
                                         (h % 2 + 1) * TC],
                        start=(c == 0), stop=(c == nch - 1))

            # exp of this many leading chunks of window j is precomputed
            # during window j-1 (fills ACT while that window is PE-bound)
            PREFETCH = [0, 0, 0, 0]

            for j in range(NJ):
                nch = 4 * (j + 1)          # causal k-chunks for this window
                # interleave next window's projection emission and the
                # previous window's output projection into the chunk loop
                proj_at = {}
                if j + 1 < NJ:
                    for part in range(5):
                        cpos = max(0, (nch * (part + 1)) // 6 - 1)
                        proj_at.setdefault(cpos, []).append(("p", part))
                if j > 0:
                    for part in range(4):
                        cpos = max(0, (nch * (part + 1)) // 5 - 2)
                        proj_at.setdefault(cpos, []).append(("y", part))
                if j + 1 < NJ:
                    for k in range(PREFETCH[j + 1]):
                        cpos = max(0, (nch * (k + 3)) // (PREFETCH[j + 1] + 3)
                                   - 1)
                        proj_at.setdefault(cpos, []).append(("e", k))

                at = [attn_sb.tile([128, TC], bf16, tag=f"attn{tau}_{j}",
                                   name=f"attn{tau}_{j}") for tau in range(2)]
                ats.append(at)
                av4 = avp.tile([128, 4 * TC], f32, tag="av", name=f"av_{j}")
                nxt = pre_ests.pop((j, 0), None)
                if nxt is None:
                    nxt = emit_scores_est(j, 0)
                for c in range(nch):
                    ests = nxt
                    # emit the next chunk's scores first so the scalar
                    # engine's exp queue is never starved behind the
                    # projection matmul blocks
                    if c + 1 < nch:
                        nxt = pre_ests.pop((j, c + 1), None)
                        if nxt is None:
                            nxt = emit_scores_est(j, c + 1)
                    evs = proj_at.get(c, ())
                    # projection matmuls go before the AV matmuls (they fill
                    # the PE while ACT runs exp); the rot-matmul + rope
                    # combine go after, hiding the u-multiply latency
                    pend = []
                    for kind, part in evs:
                        if kind == "p":
                            ctx = emit_proj_mm(j + 1, part)
                            if ctx is not None:
                                pend.append(ctx)
                    for ctx in pend:
                        emit_proj_fin(ctx)
                    emit_av(j, c, ests, av4)
                    for kind, part in evs:
                        if kind == "y":
                            emit_y(j - 1, part)
                        elif kind == "e":
                            pre_ests[(j + 1, part)] = emit_scores_est(
                                j + 1, part)

                # softmax: one scalar-engine copy per head evacuates a whole
                # AV psum bank (raw rows 0:64 + denominator row 64), so the
                # next window's AV can restart in ~2us with no DVE or bank
                # cross-serialization.  The denominator is then broadcast
                # across partitions with K=1 matmuls (ones^T @ den, ~0.2us
                # each) and the divide runs off the critical path.
                ag = []
                for h in range(HL):
                    hw = slice(h * TC, (h + 1) * TC)
                    a = small_sb.tile([65, TC], bf16, tag=f"ag{h % 2}",
                                      name=f"ag{j}_{h}")
                    # heads 0/1 on DVE, heads 2/3 on ACT: the four copies
                    # touch four different psum banks, so the two engines
                    # drain the AV psum concurrently (~1us)
                    if h < 2:
                        nc.vector.tensor_copy(a, av4[0:65, hw])
                    else:
                        nc.scalar.copy(a, av4[0:65, hw])
                    ag.append(a)
                db = [stp.tile([64, 2 * TC], f32, tag="st", name=f"db{j}_{t}")
                      for t in range(2)]
                for tau in range(2):
                    for par in range(2):
                        nc.tensor.matmul(
                            db[tau][:, par * TC:(par + 1) * TC],
                            lhsT=ones128[64:65, :],
                            rhs=ag[2 * tau + par][64:65, :],
                            start=True, stop=True,
                            tile_position=(64, 0))
                rbs = []
                for tau in range(2):
                    rb = small_sb.tile([64, 2 * TC], f32, tag="rbcast",
                                       name=f"rb{j}_{tau}")
                    rbs.append(rb)
                last = j == NJ - 1
                # divide in column halves (reciprocals split the same way,
                # so the first half's divide starts as early as possible);
                # in the epilogue each half's output-projection parts are
                # interleaved right behind it
                for half in range(2):
                    cs = slice(half * (TC // 2), (half + 1) * (TC // 2))
                    for tau in range(2):
                        for par in range(2):
                            rcs = slice(par * TC + half * (TC // 2),
                                        par * TC + (half + 1) * (TC // 2))
                            nc.vector.reciprocal_approx_fast(
                                out=rbs[tau][:, rcs], in_=db[tau][:, rcs])
                    for tau in range(2):
                        for par in range(2):
                            nc.vector.tensor_mul(
                                at[tau][64 * par:64 * par + 64, cs],
                                ag[2 * tau + par][0:64, cs],
                                rbs[tau][:, par * TC + half * (TC // 2):
                                         par * TC + (half + 1) * (TC // 2)])
                    if last:
                        # parts with th == half read the freshly divided cols
                        emit_y(j, half, copy_eng=nc.scalar)
                        emit_y(j, 2 + half, copy_eng=None)

    nc.compile()
    return nc


def _host_inputs(x, cos, sin, w_qkv, w_out):
    """Shard + lay out the full inputs for the 8 cores."""
    # natural-layout tables: row 64*l + d = cos/sin[t, d]
    cosf = np.ascontiguousarray(np.tile(cos.T, (2, 1))).astype(BF16)
    sinf = np.ascontiguousarray(np.tile(sin.T, (2, 1))).astype(BF16)

    xts = [np.ascontiguousarray(x[b].T).astype(BF16) for b in range(B)]

    # rotate_half as a stationary matmul operand: out = r2.T @ u
    r2 = np.zeros((128, 128), dtype=np.float32)
    for b in (0, 64):
        for dd in range(32):
            r2[b + 32 + dd, b + dd] = -1.0
            r2[b + dd, b + 32 + dd] = 1.0
    r2 = r2.astype(BF16)

    in_maps = []
    for core in range(8):
        b, g = divmod(core, G)
        qblk = w_qkv[:, G * g * HD:(G * g + HL) * HD]
        kblk = w_qkv[:, E + G * g * HD:E + (G * g + HL) * HD]
        vblk = w_qkv[:, 2 * E + DL * g:2 * E + DL * (g + 1)]
        wl = np.concatenate([qblk, kblk, vblk], axis=1).astype(BF16)  # (E, 768)
        wol = np.ascontiguousarray(w_out[DL * g:DL * (g + 1), :]).astype(BF16)
        in_maps.append({
            "xt": xts[b], "w": wl, "wo": wol, "cosf": cosf, "sinf": sinf,
            "r2": r2,
        })
    return in_maps


def kernel(x, cos, sin, w_qkv, w_out):
    from concourse import bass_utils

    if "nc" not in _CACHE:
        _CACHE["nc"] = _build_bass()
    nc = _CACHE["nc"]

    in_maps = _host_inputs(
        np.asarray(x, dtype=np.float32), np.asarray(cos, dtype=np.float32),
        np.asarray(sin, dtype=np.float32), np.asarray(w_qkv, dtype=np.float32),
        np.asarray(w_out, dtype=np.float32))

    res = bass_utils.run_bass_kernel_spmd(nc, in_maps, core_ids=list(range(8)))

    y = np.zeros((B, T, E), dtype=np.float32)
    for core in range(8):
        b = core // G
        y[b] += res.results[core]["y"].astype(np.float32)
    return y


# revision 36
# speedup vs baseline: 1.0530x; 1.0387x over previous
"""Causal self-attention (B=2, T=2048, E=1024, H=16) on 8 trn2 NeuronCores.

Sharding: core = b*4 + g  (b = batch index, g = head-group of 4 heads).
Each core computes its 4 heads' attention for its batch plus a partial
output projection; the host sums the 4 partials per batch.

RoPE: since sin/cos repeat across the two half-blocks (s[d+32]=s[d]),
rot(q*s) = rot(q)*s, so the kernel multiplies the projected q by sin
first (which doubles as the PSUM->SBUF staging copy) and applies
rotate_half as a single 128x128 signed-permutation matmul; q' is then
q*cos + rot(q*s) -- no duplicated rotated-weight projections.
Layout: qT tile tau rows [64l : 64l+64) = head 2tau+l dims 0..63, T on
free.  Scores are computed transposed (S^T = K Q^T, k on partitions,
K=64 contraction) so softmax exp feeds the attention matmul with no
transposes; the two heads of a tile sit on disjoint PE row-groups and
psum banks so their score matmuls run concurrently.  V carries an
appended ones column so row 64 of the attention psum is the softmax
denominator; the reciprocal is taken straight from PSUM and the divide
is fused into the PSUM->SBUF evacuation of the attention rows.  All
matmul operands bf16; fp32 accumulation; exp skips fully-invalid
diagonal columns; diagonal score matmuls and AV matmuls are trimmed to
the causally valid q-range.  A short burst of throwaway matmuls on a
memset tile runs during the weight DMA prologue to hold the PE HAM
clock-gate at full rate.
"""

import numpy as np
import ml_dtypes

BF16 = ml_dtypes.bfloat16

B, T, E = 2, 2048, 1024
H, HD = 16, 64
G = 4             # head groups (cores per batch)
HL = H // G       # heads per core
DL = HL * HD      # local qkv dim = 256
TC = 512          # T chunk (matmul moving free dim)
NJ = T // TC      # 4 q-windows
KC = 128          # k-chunk (contraction tile for attention)
NC_ = T // KC     # 16 k-chunks
SCALE = 1.0 / float(np.sqrt(HD))
WARM_MM = 26      # PE warm-up matmuls issued during the DMA prologue

_CACHE = {}


def _build_bass():
    import concourse.mybir as mybir
    import concourse.tile as tile
    from concourse import bacc

    f32 = mybir.dt.float32
    bf16 = mybir.dt.bfloat16
    EXP = mybir.ActivationFunctionType.Exp

    nc = bacc.Bacc("TRN2", target_bir_lowering=False, debug=False)
    xt_d = nc.dram_tensor("xt", [E, T], bf16, kind="ExternalInput").ap()
    w_d = nc.dram_tensor("w", [E, 3 * DL], bf16, kind="ExternalInput").ap()
    wo_d = nc.dram_tensor("wo", [DL, E], bf16, kind="ExternalInput").ap()
    cos_d = nc.dram_tensor("cosf", [128, T], bf16, kind="ExternalInput").ap()
    sin_d = nc.dram_tensor("sinf", [128, T], bf16, kind="ExternalInput").ap()
    r2_d = nc.dram_tensor("r2", [128, 128], bf16, kind="ExternalInput").ap()
    y_d = nc.dram_tensor("y", [T, E], bf16, kind="ExternalOutput").ap()

    NKK = E // KC  # 8 contraction chunks for the projections

    with tile.TileContext(nc) as tc:
        with (
            tc.tile_pool(name="consts", bufs=1) as consts,
            tc.tile_pool(name="stp", bufs=2, space="PSUM") as stp,
            tc.tile_pool(name="avp", bufs=1, space="PSUM") as avp,
            tc.tile_pool(name="tmp_sb", bufs=4) as tmp_sb,
            tc.tile_pool(name="est_sb", bufs=16) as est_sb,
            tc.tile_pool(name="attn_sb", bufs=1) as attn_sb,
            tc.tile_pool(name="ysb_p", bufs=2) as ysb_p,
            tc.tile_pool(name="small_sb", bufs=2) as small_sb,
        ):
            # ---- PE warm-up: keep the HAM clock-gate open while the
            # weight/activation DMAs land (results never read) ----
            wu = consts.tile([128, TC], bf16, tag="wu")
            nc.gpsimd.memset(wu, 0.03125)
            ones128 = consts.tile([128, 64], bf16, tag="ones128")
            nc.gpsimd.memset(ones128, 1.0)
            wups = stp.tile([128, 2 * TC], f32, tag="st", name="wup")
            for i in range(WARM_MM):
                nc.tensor.matmul(wups[:, 0:TC], lhsT=wu[:, 0:128], rhs=wu,
                                 start=True, stop=True)

            # ---- constants.  DMA order = first-window critical path;
            # the small consts ride the scalar queue so they don't delay
            # the weight/activation stream on the sync queue ----
            xt, w = [], []
            for i in range(NKK):
                tw = consts.tile([KC, 3 * DL], bf16, tag=f"w{i}", name=f"w{i}")
                nc.sync.dma_start(out=tw, in_=w_d[i * KC:(i + 1) * KC, :])
                w.append(tw)
                t = consts.tile([KC, T], bf16, tag=f"xt{i}", name=f"xt{i}")
                nc.sync.dma_start(
                    out=t[:, 0:TC], in_=xt_d[i * KC:(i + 1) * KC, 0:TC])
                xt.append(t)
                if i == 0:
                    r2 = consts.tile([128, 128], bf16, tag="r2")
                    nc.scalar.dma_start(out=r2, in_=r2_d)
                if i == 1:
                    cosf = consts.tile([128, T], bf16, tag="cosf")
                    nc.scalar.dma_start(out=cosf, in_=cos_d)
                    sinf = consts.tile([128, T], bf16, tag="sinf")
                    nc.scalar.dma_start(out=sinf, in_=sin_d)
            wo = []
            for tau in range(2):
                t = consts.tile([128, E], bf16, tag=f"wo{tau}", name=f"wo{tau}")
                nc.scalar.dma_start(
                    out=t, in_=wo_d[tau * 128:(tau + 1) * 128, :])
                wo.append(t)
            for jj in range(1, NJ):
                for i in range(NKK):
                    nc.sync.dma_start(
                        out=xt[i][:, jj * TC:(jj + 1) * TC],
                        in_=xt_d[i * KC:(i + 1) * KC, jj * TC:(jj + 1) * TC])

            # resident projection outputs (natural head-contiguous layout)
            qn = [[consts.tile([128, TC], bf16, tag=f"qn{tau}_{j}",
                               name=f"qn{tau}_{j}") for j in range(NJ)]
                  for tau in range(2)]
            kn = [[consts.tile([128, TC], bf16, tag=f"kn{tau}_{j}",
                               name=f"kn{tau}_{j}") for j in range(NJ)]
                  for tau in range(2)]
            # v tiles padded to 128 dims/head (64 v dims | ones | zeros) so
            # the AV weight loads are full 128-column FWL loads
            vsb = [consts.tile([128, HL * 128], bf16, tag=f"v{c}",
                               name=f"v{c}") for c in range(NC_)]

            def emit_proj_qk_mm(jj, base, tau, dst):
                """Projection matmuls + u = q*sin for head pair tau of
                window jj; the rot-matmul + combine are emitted later by
                emit_proj_fin so intervening PE work hides the DVE wait."""
                js = slice(jj * TC, (jj + 1) * TC)
                ps = stp.tile([128, 2 * TC], f32, tag="st",
                              name=f"pqk{base}_{tau}_{jj}")
                cc = base + 128 * tau
                for kk in range(NKK):
                    nc.tensor.matmul(
                        ps[:, 0:TC],
                        lhsT=w[kk][:, cc:cc + 128],
                        rhs=xt[kk][:, js],
                        start=(kk == 0), stop=(kk == NKK - 1))
                u = tmp_sb.tile([128, TC], bf16, tag="ropeU",
                                name=f"ru{base}_{tau}_{jj}")
                nc.vector.tensor_mul(u, ps[:, 0:TC], sinf[:, js])
                return (ps, u, jj, tau, dst)

            def emit_proj_fin(ctx):
                ps, u, jj, tau, dst = ctx
                js = slice(jj * TC, (jj + 1) * TC)
                nc.tensor.matmul(ps[:, TC:2 * TC], lhsT=r2, rhs=u,
                                 start=True, stop=True)
                qk = 0 if dst is qn else 1
                ta = tmp_sb.tile([128, TC], f32, tag="ropeA",
                                 name=f"ra{qk}_{tau}_{jj}")
                nc.vector.tensor_mul(ta, ps[:, 0:TC], cosf[:, js])
                nc.vector.tensor_add(dst[tau][jj], ta, ps[:, TC:2 * TC])

            def emit_proj_v(jj):
                js0 = jj * (TC // KC)
                ps = stp.tile([128, 4 * DL], f32, tag="st", name=f"pv{jj}")
                for tt in range(TC // KC):
                    c = js0 + tt
                    for kk in range(NKK):
                        nc.tensor.matmul(
                            ps[:, tt * DL:(tt + 1) * DL],
                            lhsT=xt[kk][:, c * KC:(c + 1) * KC],
                            rhs=w[kk][:, 2 * DL:3 * DL],
                            start=(kk == 0), stop=(kk == NKK - 1))
                for tt in range(TC // KC):
                    c = js0 + tt
                    vv = vsb[c].rearrange("p (h d) -> p h d", h=HL)
                    nc.gpsimd.memset(vv[:, :, 64:128], 0.0)
                    nc.gpsimd.memset(vv[:, :, 64:65], 1.0)
                    nc.vector.tensor_copy(
                        vv[:, :, 0:64],
                        ps[:, tt * DL:(tt + 1) * DL].rearrange(
                            "p (h d) -> p h d", h=HL))

            def emit_proj_mm(jj, part):
                if part == 0:
                    return emit_proj_qk_mm(jj, 0, 0, qn)
                elif part == 1:
                    return emit_proj_qk_mm(jj, 0, 1, qn)
                elif part == 2:
                    return emit_proj_qk_mm(jj, DL, 0, kn)
                elif part == 3:
                    return emit_proj_qk_mm(jj, DL, 1, kn)
                emit_proj_v(jj)
                return None

            def emit_y(jj, part, copy_eng=None):
                n, th = divmod(part, 2)
                at = ats[jj]
                yp = stp.tile([128, 2 * TC], f32, tag="st",
                              name=f"yp{jj}_{n}_{th}")
                for ti in range(2):
                    tt = 2 * th + ti
                    for tau in range(2):
                        nc.tensor.matmul(
                            yp[:, ti * TC:(ti + 1) * TC],
                            lhsT=at[tau][:, tt * KC:(tt + 1) * KC],
                            rhs=wo[tau][:, n * TC:(n + 1) * TC],
                            start=(tau == 0), stop=(tau == 1))
                ys = ysb_p.tile([128, 2 * TC], bf16, tag="y",
                                name=f"ys{jj}_{n}_{th}")
                if copy_eng is nc.scalar:
                    nc.scalar.copy(ys, yp)
                else:
                    nc.vector.tensor_copy(ys, yp)
                nc.sync.dma_start(
                    out=y_d[jj * TC + th * 2 * KC:
                            jj * TC + (th + 1) * 2 * KC,
                            n * TC:(n + 1) * TC].rearrange(
                                "(a p) c -> p a c", p=128),
                    in_=ys.rearrange("p (a c) -> p a c", a=2))

            # prologue: projections for window 0, software-pipelined so each
            # rot-matmul waits behind the next projection's matmuls
            pend = []
            for part in range(5):
                ctx = emit_proj_mm(0, part)
                if len(pend) > 0:
                    emit_proj_fin(pend.pop(0))
                if ctx is not None:
                    pend.append(ctx)
            while pend:
                emit_proj_fin(pend.pop(0))
            ats = []
            pre_ests = {}              # (j, c) -> [est01, est23] prefetched

            def emit_scores_est(j, c):
                """Score pair + exp for chunk c of window j; returns the two
                est tiles.  Per-tau emission so the two score psum slots
                double-buffer across half-chunks."""
                nch = 4 * (j + 1)
                d = c - 4 * j
                qoff = KC * d if d > 0 else 0
                ests = []
                for tau in range(2):
                    stt = stp.tile([128, 2 * TC], f32, tag="st",
                                   name=f"st{j}_{c}_{tau}")
                    for ll in range(2):
                        w0c = ll * TC
                        nc.tensor.matmul(
                            stt[:, w0c + qoff:w0c + TC],
                            lhsT=kn[tau][c // 4][
                                64 * ll:64 * ll + 64,
                                (c % 4) * KC:(c % 4 + 1) * KC],
                            rhs=qn[tau][j][64 * ll:64 * ll + 64, qoff:],
                            start=True, stop=True,
                            tile_position=(64 * ll, 0))
                    est = est_sb.tile([128, 2 * TC], bf16, tag="est",
                                      name=f"est{j}_{c}_{tau}")
                    if qoff:
                        nc.scalar.activation(
                            est.rearrange("p (w c) -> p w c", w=2)[
                                :, :, qoff:],
                            stt.rearrange("p (w c) -> p w c", w=2)[
                                :, :, qoff:],
                            EXP, scale=SCALE)
                    else:
                        nc.scalar.activation(est, stt, EXP, scale=SCALE)
                    if d >= 0:
                        # zero the upper-triangular (future) part of the
                        # diagonal KC-block, on the (otherwise idle) gpsimd
                        for ll in range(2):
                            bs = slice(ll * TC + KC * d,
                                       ll * TC + KC * (d + 1))
                            nc.gpsimd.affine_select(
                                out=est[:, bs], in_=est[:, bs],
                                compare_op=mybir.AluOpType.is_ge, fill=0.0,
                                base=0, pattern=[[1, KC]],
                                channel_multiplier=-1)
                    ests.append(est)
                return ests

            def emit_av(j, c, ests, av4):
                nch = 4 * (j + 1)
                d = c - 4 * j
                coff = KC * d if d > 0 else 0
                for h in range(HL):
                    nc.tensor.matmul(
                        av4[0:128, h * TC + coff:(h + 1) * TC],
                        lhsT=vsb[c][:, 128 * h:128 * h + 128],
                        rhs=ests[h // 2][:, (h % 2) * TC + coff:
                                         (h % 2 + 1) * TC],
                        start=(c == 0), stop=(c == nch - 1))

            # exp of this many leading chunks of window j is precomputed
            # during window j-1 (fills ACT while that window is PE-bound)
            PREFETCH = [0, 0, 0, 0]

            for j in range(NJ):
                nch = 4 * (j + 1)          # causal k-chunks for this window
                # interleave next window's projection emission and the
                # previous window's output projection into the chunk loop
                proj_at = {}
                if j + 1 < NJ:
                    for part in range(5):
                        cpos = max(0, (nch * (part + 1)) // 6 - 1)
                        proj_at.setdefault(cpos, []).append(("p", part))
                if j > 0:
                    for part in range(4):
                        cpos = max(0, (nch * (part + 1)) // 5 - 2)
                        proj_at.setdefault(cpos, []).append(("y", part))
                if j + 1 < NJ:
                    for k in range(PREFETCH[j + 1]):
                        cpos = max(0, (nch * (k + 3)) // (PREFETCH[j + 1] + 3)
                                   - 1)
                        proj_at.setdefault(cpos, []).append(("e", k))

                at = [attn_sb.tile([128, TC], bf16, tag=f"attn{tau}_{j}",
                                   name=f"attn{tau}_{j}") for tau in range(2)]
                ats.append(at)
                av4 = avp.tile([128, 4 * TC], f32, tag="av", name=f"av_{j}")
                nxt = pre_ests.pop((j, 0), None)
                if nxt is None:
                    nxt = emit_scores_est(j, 0)
                for c in range(nch):
                    ests = nxt
                    # emit the next chunk's scores first so the scalar
                    # engine's exp queue is never starved behind the
                    # projection matmul blocks
                    if c + 1 < nch:
                        nxt = pre_ests.pop((j, c + 1), None)
                        if nxt is None:
                            nxt = emit_scores_est(j, c + 1)
                    evs = proj_at.get(c, ())
                    # projection matmuls go before the AV matmuls (they fill
                    # the PE while ACT runs exp); the rot-matmul + rope
                    # combine go after, hiding the u-multiply latency
                    pend = []
                    for kind, part in evs:
                        if kind == "p":
                            ctx = emit_proj_mm(j + 1, part)
                            if ctx is not None:
                                pend.append(ctx)
                    for ctx in pend:
                        emit_proj_fin(ctx)
                    emit_av(j, c, ests, av4)
                    for kind, part in evs:
                        if kind == "y":
                            emit_y(j - 1, part)
                        elif kind == "e":
                            pre_ests[(j + 1, part)] = emit_scores_est(
                                j + 1, part)

                # softmax: one scalar-engine copy per head evacuates a whole
                # AV psum bank (raw rows 0:64 + denominator row 64), so the
                # next window's AV can restart in ~2us with no DVE or bank
                # cross-serialization.  The denominator is then broadcast
                # across partitions with K=1 matmuls (ones^T @ den, ~0.2us
                # each) and the divide runs off the critical path.
                ag = []
                for h in range(HL):
                    hw = slice(h * TC, (h + 1) * TC)
                    a = small_sb.tile([65, TC], bf16, tag=f"ag{h % 2}",
                                      name=f"ag{j}_{h}")
                    # heads 0/1 on DVE, heads 2/3 on ACT: the four copies
                    # touch four different psum banks, so the two engines
                    # drain the AV psum concurrently (~1us)
                    if h < 2:
                        nc.vector.tensor_copy(a, av4[0:65, hw])
                    else:
                        nc.scalar.copy(a, av4[0:65, hw])
                    ag.append(a)
                db = [stp.tile([64, 2 * TC], f32, tag="st", name=f"db{j}_{t}")
                      for t in range(2)]
                for tau in range(2):
                    for par in range(2):
                        nc.tensor.matmul(
                            db[tau][:, par * TC:(par + 1) * TC],
                            lhsT=ones128[64:65, :],
                            rhs=ag[2 * tau + par][64:65, :],
                            start=True, stop=True,
                            tile_position=(64, 0))
                rbs = []
                for tau in range(2):
                    rb = small_sb.tile([64, 2 * TC], f32, tag="rbcast",
                                       name=f"rb{j}_{tau}")
                    nc.vector.reciprocal_approx_fast(out=rb, in_=db[tau])
                    rbs.append(rb)
                last = j == NJ - 1
                # divide in column halves; in the epilogue each half's
                # output-projection parts are interleaved right behind it
                for half in range(2):
                    cs = slice(half * (TC // 2), (half + 1) * (TC // 2))
                    for tau in range(2):
                        for par in range(2):
                            nc.vector.tensor_mul(
                                at[tau][64 * par:64 * par + 64, cs],
                                ag[2 * tau + par][0:64, cs],
                                rbs[tau][:, par * TC + half * (TC // 2):
                                         par * TC + (half + 1) * (TC // 2)])
                    if last:
                        # parts with th == half read the freshly divided cols
                        emit_y(j, half, copy_eng=nc.scalar)
                        emit_y(j, 2 + half, copy_eng=None)

    nc.compile()
    return nc


def _host_inputs(x, cos, sin, w_qkv, w_out):
    """Shard + lay out the full inputs for the 8 cores."""
    # natural-layout tables: row 64*l + d = cos/sin[t, d]
    cosf = np.ascontiguousarray(np.tile(cos.T, (2, 1))).astype(BF16)
    sinf = np.ascontiguousarray(np.tile(sin.T, (2, 1))).astype(BF16)

    xts = [np.ascontiguousarray(x[b].T).astype(BF16) for b in range(B)]

    # rotate_half as a stationary matmul operand: out = r2.T @ u
    r2 = np.zeros((128, 128), dtype=np.float32)
    for b in (0, 64):
        for dd in range(32):
            r2[b + 32 + dd, b + dd] = -1.0
            r2[b + dd, b + 32 + dd] = 1.0
    r2 = r2.astype(BF16)

    in_maps = []
    for core in range(8):
        b, g = divmod(core, G)
        qblk = w_qkv[:, G * g * HD:(G * g + HL) * HD]
        kblk = w_qkv[:, E + G * g * HD:E + (G * g + HL) * HD]
        vblk = w_qkv[:, 2 * E + DL * g:2 * E + DL * (g + 1)]
        wl = np.concatenate([qblk, kblk, vblk], axis=1).astype(BF16)  # (E, 768)
        wol = np.ascontiguousarray(w_out[DL * g:DL * (g + 1), :]).astype(BF16)
        in_maps.append({
            "xt": xts[b], "w": wl, "wo": wol, "cosf": cosf, "sinf": sinf,
            "r2": r2,
        })
    return in_maps


def kernel(x, cos, sin, w_qkv, w_out):
    from concourse import bass_utils

    if "nc" not in _CACHE:
        _CACHE["nc"] = _build_bass()
    nc = _CACHE["nc"]

    in_maps = _host_inputs(
        np.asarray(x, dtype=np.float32), np.asarray(cos, dtype=np.float32),
        np.asarray(sin, dtype=np.float32), np.asarray(w_qkv, dtype=np.float32),
        np.asarray(w_out, dtype=np.float32))

    res = bass_utils.run_bass_kernel_spmd(nc, in_maps, core_ids=list(range(8)))

    y = np.zeros((B, T, E), dtype=np.float32)
    for core in range(8):
        b = core // G
        y[b] += res.results[core]["y"].astype(np.float32)
    return y


# revision 38
# speedup vs baseline: 1.1399x; 1.0826x over previous
"""Causal self-attention (B=2, T=2048, E=1024, H=16) on 8 trn2 NeuronCores.

Sharding: core = b*4 + g  (b = batch index, g = head-group of 4 heads).
Each core computes its 4 heads' attention for its batch plus a partial
output projection; the host sums the 4 partials per batch.

RoPE: since sin/cos repeat across the two half-blocks (s[d+32]=s[d]),
rot(q*s) = rot(q)*s, so the kernel multiplies the projected q by sin
first (which doubles as the PSUM->SBUF staging copy) and applies
rotate_half as a single 128x128 signed-permutation matmul; q' is then
q*cos + rot(q*s) -- no duplicated rotated-weight projections.
Layout: qT tile tau rows [64l : 64l+64) = head 2tau+l dims 0..63, T on
free.  Scores are computed transposed (S^T = K Q^T, k on partitions,
K=64 contraction) so softmax exp feeds the attention matmul with no
transposes; the two heads of a tile sit on disjoint PE row-groups and
psum banks so their score matmuls run concurrently.  V carries an
appended ones column so row 64 of the attention psum is the softmax
denominator; the reciprocal is taken straight from PSUM and the divide
is fused into the PSUM->SBUF evacuation of the attention rows.  All
matmul operands bf16; fp32 accumulation; exp skips fully-invalid
diagonal columns; diagonal score matmuls and AV matmuls are trimmed to
the causally valid q-range.  A short burst of throwaway matmuls on a
memset tile runs during the weight DMA prologue to hold the PE HAM
clock-gate at full rate.
"""

import numpy as np
import ml_dtypes

BF16 = ml_dtypes.bfloat16
F8 = ml_dtypes.float8_e4m3
WS = 64.0         # host-side scale on the fp8 qkv weights

B, T, E = 2, 2048, 1024
H, HD = 16, 64
G = 4             # head groups (cores per batch)
HL = H // G       # heads per core
DL = HL * HD      # local qkv dim = 256
TC = 512          # T chunk (matmul moving free dim)
NJ = T // TC      # 4 q-windows
KC = 128          # k-chunk (contraction tile for attention)
NC_ = T // KC     # 16 k-chunks
SCALE = 1.0 / float(np.sqrt(HD))
ESCALE = SCALE / (64.0 * 64.0)
WARM_MM = 26      # PE warm-up matmuls issued during the DMA prologue

_CACHE = {}


def _build_bass():
    import concourse.mybir as mybir
    import concourse.tile as tile
    from concourse import bacc

    f32 = mybir.dt.float32
    bf16 = mybir.dt.bfloat16
    EXP = mybir.ActivationFunctionType.Exp

    f8 = mybir.dt.float8e4
    DR = mybir.MatmulPerfMode.DoubleRow

    nc = bacc.Bacc("TRN2", target_bir_lowering=False, debug=False)
    # qkv weights and x^T in fp8 (e4m3), packed [p, i, m] per 256-row
    # chunk (contraction pair k = 128*i + p) for DoubleRow matmuls
    xt_d = nc.dram_tensor("xt8", [E // 2, 2 * T], f8,
                          kind="ExternalInput").ap()
    w_d = nc.dram_tensor("w8", [E // 2, 2 * 2 * DL], f8,
                         kind="ExternalInput").ap()
    # v path stays bf16 (its quantization error would pass straight
    # through to the output; the q/k error is softmax-dampened)
    xtb_d = nc.dram_tensor("xtb", [E, T], bf16, kind="ExternalInput").ap()
    wv_d = nc.dram_tensor("wv", [E, DL], bf16, kind="ExternalInput").ap()
    wo_d = nc.dram_tensor("wo", [DL, E], bf16, kind="ExternalInput").ap()
    cos_d = nc.dram_tensor("cosf", [128, T], bf16, kind="ExternalInput").ap()
    sin_d = nc.dram_tensor("sinf", [128, T], bf16, kind="ExternalInput").ap()
    r2_d = nc.dram_tensor("r2", [128, 128], bf16, kind="ExternalInput").ap()
    y_d = nc.dram_tensor("y", [T, E], bf16, kind="ExternalOutput").ap()

    NKK = E // KC  # 8 contraction chunks for the projections

    with tile.TileContext(nc) as tc:
        with (
            tc.tile_pool(name="consts", bufs=1) as consts,
            tc.tile_pool(name="stp", bufs=2, space="PSUM") as stp,
            tc.tile_pool(name="avp", bufs=1, space="PSUM") as avp,
            tc.tile_pool(name="tmp_sb", bufs=4) as tmp_sb,
            tc.tile_pool(name="est_sb", bufs=16) as est_sb,
            tc.tile_pool(name="attn_sb", bufs=1) as attn_sb,
            tc.tile_pool(name="ysb_p", bufs=2) as ysb_p,
            tc.tile_pool(name="small_sb", bufs=2) as small_sb,
        ):
            # ---- PE warm-up: keep the HAM clock-gate open while the
            # weight/activation DMAs land (results never read) ----
            wu = consts.tile([128, TC], bf16, tag="wu")
            nc.gpsimd.memset(wu, 0.03125)
            ones128 = consts.tile([128, 64], bf16, tag="ones128")
            nc.gpsimd.memset(ones128, 1.0)
            wups = stp.tile([128, 2 * TC], f32, tag="st", name="wup")
            for i in range(WARM_MM):
                nc.tensor.matmul(wups[:, 0:TC], lhsT=wu[:, 0:128], rhs=wu,
                                 start=True, stop=True)

            # ---- constants.  DMA order = first-window critical path;
            # the small consts ride the scalar queue so they don't delay
            # the weight/activation stream on the sync queue ----
            NK2 = NKK // 2  # 4 DoubleRow chunks of 256 contraction rows
            xt, w, xtb, wv = [], [], [], []
            for i in range(NK2):
                tw = consts.tile([KC, 2 * 2 * DL], f8, tag=f"w{i}",
                                 name=f"w{i}")
                nc.sync.dma_start(out=tw, in_=w_d[i * KC:(i + 1) * KC, :])
                w.append(tw.rearrange("p (i m) -> p i m", i=2))
                t = consts.tile([KC, 2 * T], f8, tag=f"xt{i}", name=f"xt{i}")
                t3 = t.rearrange("p (i t) -> p i t", i=2)
                nc.sync.dma_start(
                    out=t3[:, :, 0:TC],
                    in_=xt_d[i * KC:(i + 1) * KC, :].rearrange(
                        "p (i t) -> p i t", i=2)[:, :, 0:TC])
                xt.append(t3)
                if i == 0:
                    r2 = consts.tile([128, 128], bf16, tag="r2")
                    nc.scalar.dma_start(out=r2, in_=r2_d)
                    cosf = consts.tile([128, T], bf16, tag="cosf")
                    nc.scalar.dma_start(out=cosf, in_=cos_d)
                    sinf = consts.tile([128, T], bf16, tag="sinf")
                    nc.scalar.dma_start(out=sinf, in_=sin_d)
            for i in range(NKK):
                tv = consts.tile([KC, DL], bf16, tag=f"wv{i}", name=f"wv{i}")
                nc.sync.dma_start(out=tv, in_=wv_d[i * KC:(i + 1) * KC, :])
                wv.append(tv)
                tb = consts.tile([KC, T], bf16, tag=f"xtb{i}", name=f"xtb{i}")
                nc.sync.dma_start(
                    out=tb[:, 0:TC], in_=xtb_d[i * KC:(i + 1) * KC, 0:TC])
                xtb.append(tb)
            wo = []
            for tau in range(2):
                t = consts.tile([128, E], bf16, tag=f"wo{tau}", name=f"wo{tau}")
                nc.scalar.dma_start(
                    out=t, in_=wo_d[tau * 128:(tau + 1) * 128, :])
                wo.append(t)
            for jj in range(1, NJ):
                for i in range(NK2):
                    nc.sync.dma_start(
                        out=xt[i][:, :, jj * TC:(jj + 1) * TC],
                        in_=xt_d[i * KC:(i + 1) * KC, :].rearrange(
                            "p (i t) -> p i t", i=2)[:, :,
                                                     jj * TC:(jj + 1) * TC])
                for i in range(NKK):
                    nc.sync.dma_start(
                        out=xtb[i][:, jj * TC:(jj + 1) * TC],
                        in_=xtb_d[i * KC:(i + 1) * KC,
                                  jj * TC:(jj + 1) * TC])

            # resident projection outputs (natural head-contiguous layout)
            qn = [[consts.tile([128, TC], bf16, tag=f"qn{tau}_{j}",
                               name=f"qn{tau}_{j}") for j in range(NJ)]
                  for tau in range(2)]
            kn = [[consts.tile([128, TC], bf16, tag=f"kn{tau}_{j}",
                               name=f"kn{tau}_{j}") for j in range(NJ)]
                  for tau in range(2)]
            # v tiles padded to 128 dims/head (64 v dims | ones | zeros) so
            # the AV weight loads are full 128-column FWL loads
            vsb = [consts.tile([128, HL * 128], bf16, tag=f"v{c}",
                               name=f"v{c}") for c in range(NC_)]

            def emit_proj_qk_mm(jj, base, tau, dst):
                """Projection matmuls + u = q*sin for head pair tau of
                window jj; the rot-matmul + combine are emitted later by
                emit_proj_fin so intervening PE work hides the DVE wait."""
                js = slice(jj * TC, (jj + 1) * TC)
                ps = stp.tile([128, 2 * TC], f32, tag="st",
                              name=f"pqk{base}_{tau}_{jj}")
                cc = base + 128 * tau
                for kk in range(NKK // 2):
                    nc.tensor.matmul(
                        ps[:, 0:TC],
                        lhsT=w[kk][:, :, cc:cc + 128],
                        rhs=xt[kk][:, :, js],
                        start=(kk == 0), stop=(kk == NKK // 2 - 1),
                        perf_mode=DR)
                u = tmp_sb.tile([128, TC], bf16, tag="ropeU",
                                name=f"ru{base}_{tau}_{jj}")
                nc.vector.tensor_mul(u, ps[:, 0:TC], sinf[:, js])
                return (ps, u, jj, tau, dst)

            def emit_proj_fin(ctx):
                ps, u, jj, tau, dst = ctx
                js = slice(jj * TC, (jj + 1) * TC)
                nc.tensor.matmul(ps[:, TC:2 * TC], lhsT=r2, rhs=u,
                                 start=True, stop=True)
                qk = 0 if dst is qn else 1
                ta = tmp_sb.tile([128, TC], f32, tag="ropeA",
                                 name=f"ra{qk}_{tau}_{jj}")
                nc.vector.tensor_mul(ta, ps[:, 0:TC], cosf[:, js])
                nc.vector.tensor_add(dst[tau][jj], ta, ps[:, TC:2 * TC])

            def emit_proj_v(jj):
                js0 = jj * (TC // KC)
                ps = stp.tile([128, 4 * DL], f32, tag="st", name=f"pv{jj}")
                for tt in range(TC // KC):
                    c = js0 + tt
                    for kk in range(NKK):
                        nc.tensor.matmul(
                            ps[:, tt * DL:(tt + 1) * DL],
                            lhsT=xtb[kk][:, c * KC:(c + 1) * KC],
                            rhs=wv[kk],
                            start=(kk == 0), stop=(kk == NKK - 1))
                for tt in range(TC // KC):
                    c = js0 + tt
                    vv = vsb[c].rearrange("p (h d) -> p h d", h=HL)
                    nc.gpsimd.memset(vv[:, :, 64:128], 0.0)
                    nc.gpsimd.memset(vv[:, :, 64:65], 1.0)
                    nc.vector.tensor_copy(
                        vv[:, :, 0:64],
                        ps[:, tt * DL:(tt + 1) * DL].rearrange(
                            "p (h d) -> p h d", h=HL))

            def emit_proj_mm(jj, part):
                if part == 0:
                    return emit_proj_qk_mm(jj, 0, 0, qn)
                elif part == 1:
                    return emit_proj_qk_mm(jj, 0, 1, qn)
                elif part == 2:
                    return emit_proj_qk_mm(jj, DL, 0, kn)
                elif part == 3:
                    return emit_proj_qk_mm(jj, DL, 1, kn)
                emit_proj_v(jj)
                return None

            def emit_y(jj, part, copy_eng=None):
                n, th = divmod(part, 2)
                at = ats[jj]
                yp = stp.tile([128, 2 * TC], f32, tag="st",
                              name=f"yp{jj}_{n}_{th}")
                for ti in range(2):
                    tt = 2 * th + ti
                    for tau in range(2):
                        nc.tensor.matmul(
                            yp[:, ti * TC:(ti + 1) * TC],
                            lhsT=at[tau][:, tt * KC:(tt + 1) * KC],
                            rhs=wo[tau][:, n * TC:(n + 1) * TC],
                            start=(tau == 0), stop=(tau == 1))
                ys = ysb_p.tile([128, 2 * TC], bf16, tag="y",
                                name=f"ys{jj}_{n}_{th}")
                if copy_eng is nc.scalar:
                    nc.scalar.copy(ys, yp)
                else:
                    nc.vector.tensor_copy(ys, yp)
                nc.sync.dma_start(
                    out=y_d[jj * TC + th * 2 * KC:
                            jj * TC + (th + 1) * 2 * KC,
                            n * TC:(n + 1) * TC].rearrange(
                                "(a p) c -> p a c", p=128),
                    in_=ys.rearrange("p (a c) -> p a c", a=2))

            # prologue: projections for window 0, software-pipelined so each
            # rot-matmul waits behind the next projection's matmuls
            pend = []
            for part in range(5):
                ctx = emit_proj_mm(0, part)
                if len(pend) > 0:
                    emit_proj_fin(pend.pop(0))
                if ctx is not None:
                    pend.append(ctx)
            while pend:
                emit_proj_fin(pend.pop(0))
            ats = []
            pre_ests = {}              # (j, c) -> [est01, est23] prefetched

            def emit_scores_est(j, c):
                """Score pair + exp for chunk c of window j; returns the two
                est tiles.  Per-tau emission so the two score psum slots
                double-buffer across half-chunks."""
                nch = 4 * (j + 1)
                d = c - 4 * j
                qoff = KC * d if d > 0 else 0
                ests = []
                for tau in range(2):
                    stt = stp.tile([128, 2 * TC], f32, tag="st",
                                   name=f"st{j}_{c}_{tau}")
                    for ll in range(2):
                        w0c = ll * TC
                        nc.tensor.matmul(
                            stt[:, w0c + qoff:w0c + TC],
                            lhsT=kn[tau][c // 4][
                                64 * ll:64 * ll + 64,
                                (c % 4) * KC:(c % 4 + 1) * KC],
                            rhs=qn[tau][j][64 * ll:64 * ll + 64, qoff:],
                            start=True, stop=True,
                            tile_position=(64 * ll, 0))
                    est = est_sb.tile([128, 2 * TC], bf16, tag="est",
                                      name=f"est{j}_{c}_{tau}")
                    if qoff:
                        nc.scalar.activation(
                            est.rearrange("p (w c) -> p w c", w=2)[
                                :, :, qoff:],
                            stt.rearrange("p (w c) -> p w c", w=2)[
                                :, :, qoff:],
                            EXP, scale=ESCALE)
                    else:
                        nc.scalar.activation(est, stt, EXP, scale=ESCALE)
                    if d >= 0:
                        # zero the upper-triangular (future) part of the
                        # diagonal KC-block, on the (otherwise idle) gpsimd
                        for ll in range(2):
                            bs = slice(ll * TC + KC * d,
                                       ll * TC + KC * (d + 1))
                            nc.gpsimd.affine_select(
                                out=est[:, bs], in_=est[:, bs],
                                compare_op=mybir.AluOpType.is_ge, fill=0.0,
                                base=0, pattern=[[1, KC]],
                                channel_multiplier=-1)
                    ests.append(est)
                return ests

            def emit_av(j, c, ests, av4):
                nch = 4 * (j + 1)
                d = c - 4 * j
                coff = KC * d if d > 0 else 0
                for h in range(HL):
                    nc.tensor.matmul(
                        av4[0:128, h * TC + coff:(h + 1) * TC],
                        lhsT=vsb[c][:, 128 * h:128 * h + 128],
                        rhs=ests[h // 2][:, (h % 2) * TC + coff:
                                         (h % 2 + 1) * TC],
                        start=(c == 0), stop=(c == nch - 1))

            # exp of this many leading chunks of window j is precomputed
            # during window j-1 (fills ACT while that window is PE-bound)
            PREFETCH = [0, 0, 0, 0]

            for j in range(NJ):
                nch = 4 * (j + 1)          # causal k-chunks for this window
                # interleave next window's projection emission and the
                # previous window's output projection into the chunk loop
                proj_at = {}
                if j + 1 < NJ:
                    for part in range(5):
                        cpos = max(0, (nch * (part + 1)) // 6 - 1)
                        proj_at.setdefault(cpos, []).append(("p", part))
                if j > 0:
                    for part in range(4):
                        cpos = max(0, (nch * (part + 1)) // 5 - 2)
                        proj_at.setdefault(cpos, []).append(("y", part))
                if j + 1 < NJ:
                    for k in range(PREFETCH[j + 1]):
                        cpos = max(0, (nch * (k + 3)) // (PREFETCH[j + 1] + 3)
                                   - 1)
                        proj_at.setdefault(cpos, []).append(("e", k))

                at = [attn_sb.tile([128, TC], bf16, tag=f"attn{tau}_{j}",
                                   name=f"attn{tau}_{j}") for tau in range(2)]
                ats.append(at)
                av4 = avp.tile([128, 4 * TC], f32, tag="av", name=f"av_{j}")
                nxt = pre_ests.pop((j, 0), None)
                if nxt is None:
                    nxt = emit_scores_est(j, 0)
                for c in range(nch):
                    ests = nxt
                    # emit the next chunk's scores first so the scalar
                    # engine's exp queue is never starved behind the
                    # projection matmul blocks
                    if c + 1 < nch:
                        nxt = pre_ests.pop((j, c + 1), None)
                        if nxt is None:
                            nxt = emit_scores_est(j, c + 1)
                    evs = proj_at.get(c, ())
                    # projection matmuls go before the AV matmuls (they fill
                    # the PE while ACT runs exp); the rot-matmul + rope
                    # combine go after, hiding the u-multiply latency
                    pend = []
                    for kind, part in evs:
                        if kind == "p":
                            ctx = emit_proj_mm(j + 1, part)
                            if ctx is not None:
                                pend.append(ctx)
                    for ctx in pend:
                        emit_proj_fin(ctx)
                    emit_av(j, c, ests, av4)
                    for kind, part in evs:
                        if kind == "y":
                            emit_y(j - 1, part)
                        elif kind == "e":
                            pre_ests[(j + 1, part)] = emit_scores_est(
                                j + 1, part)

                # softmax: one scalar-engine copy per head evacuates a whole
                # AV psum bank (raw rows 0:64 + denominator row 64), so the
                # next window's AV can restart in ~2us with no DVE or bank
                # cross-serialization.  The denominator is then broadcast
                # across partitions with K=1 matmuls (ones^T @ den, ~0.2us
                # each) and the divide runs off the critical path.
                ag = []
                for h in range(HL):
                    hw = slice(h * TC, (h + 1) * TC)
                    a = small_sb.tile([65, TC], bf16, tag=f"ag{h % 2}",
                                      name=f"ag{j}_{h}")
                    # heads 0/1 on DVE, heads 2/3 on ACT: the four copies
                    # touch four different psum banks, so the two engines
                    # drain the AV psum concurrently (~1us)
                    if h < 2:
                        nc.vector.tensor_copy(a, av4[0:65, hw])
                    else:
                        nc.scalar.copy(a, av4[0:65, hw])
                    ag.append(a)
                db = [stp.tile([64, 2 * TC], f32, tag="st", name=f"db{j}_{t}")
                      for t in range(2)]
                for tau in range(2):
                    for par in range(2):
                        nc.tensor.matmul(
                            db[tau][:, par * TC:(par + 1) * TC],
                            lhsT=ones128[64:65, :],
                            rhs=ag[2 * tau + par][64:65, :],
                            start=True, stop=True,
                            tile_position=(64, 0))
                rbs = []
                for tau in range(2):
                    rb = small_sb.tile([64, 2 * TC], f32, tag="rbcast",
                                       name=f"rb{j}_{tau}")
                    nc.vector.reciprocal_approx_fast(out=rb, in_=db[tau])
                    rbs.append(rb)
                last = j == NJ - 1
                # divide in column halves; in the epilogue each half's
                # output-projection parts are interleaved right behind it
                for half in range(2):
                    cs = slice(half * (TC // 2), (half + 1) * (TC // 2))
                    for tau in range(2):
                        for par in range(2):
                            nc.vector.tensor_mul(
                                at[tau][64 * par:64 * par + 64, cs],
                                ag[2 * tau + par][0:64, cs],
                                rbs[tau][:, par * TC + half * (TC // 2):
                                         par * TC + (half + 1) * (TC // 2)])
                    if last:
                        # parts with th == half read the freshly divided cols
                        emit_y(j, half, copy_eng=nc.scalar)
                        emit_y(j, 2 + half, copy_eng=None)

    nc.compile()
    return nc


def _host_inputs(x, cos, sin, w_qkv, w_out):
    """Shard + lay out the full inputs for the 8 cores."""
    # natural-layout tables: row 64*l + d = cos/sin[t, d]
    cosf = np.ascontiguousarray(np.tile(cos.T, (2, 1))).astype(BF16)
    sinf = np.ascontiguousarray(np.tile(sin.T, (2, 1))).astype(BF16)

    def pack_dr(a):
        """(E, M) -> fp8 [E//2, 2*M] with pair k = 128*i + p per
        256-row chunk, [p, i, m] layout."""
        E_, M_ = a.shape
        out = np.empty((E_ // 2, 2 * M_), dtype=F8)
        for c2 in range(E_ // 256):
            blk = a[256 * c2:256 * (c2 + 1), :].reshape(2, 128, M_)
            out[128 * c2:128 * (c2 + 1), :] = (
                blk.transpose(1, 0, 2).reshape(128, 2 * M_).astype(F8))
        return out

    xt8s = [pack_dr(x[b].T) for b in range(B)]
    xtbs = [np.ascontiguousarray(x[b].T).astype(BF16) for b in range(B)]

    # rotate_half as a stationary matmul operand: out = r2.T @ u
    r2 = np.zeros((128, 128), dtype=np.float32)
    for b in (0, 64):
        for dd in range(32):
            r2[b + 32 + dd, b + dd] = -1.0
            r2[b + dd, b + 32 + dd] = 1.0
    r2 = r2.astype(BF16)

    in_maps = []
    for core in range(8):
        b, g = divmod(core, G)
        qblk = w_qkv[:, G * g * HD:(G * g + HL) * HD]
        kblk = w_qkv[:, E + G * g * HD:E + (G * g + HL) * HD]
        vblk = w_qkv[:, 2 * E + DL * g:2 * E + DL * (g + 1)]
        wl = np.concatenate([qblk, kblk], axis=1)             # (E, 512)
        w8l = pack_dr(np.asarray(wl) * WS)
        wvl = np.ascontiguousarray(vblk).astype(BF16)
        wol = np.ascontiguousarray(
            w_out[DL * g:DL * (g + 1), :]).astype(BF16)
        in_maps.append({
            "xt8": xt8s[b], "w8": w8l, "wv": wvl, "xtb": xtbs[b],
            "wo": wol, "cosf": cosf, "sinf": sinf, "r2": r2,
        })
    return in_maps


def kernel(x, cos, sin, w_qkv, w_out):
    from concourse import bass_utils

    if "nc" not in _CACHE:
        _CACHE["nc"] = _build_bass()
    nc = _CACHE["nc"]

    in_maps = _host_inputs(
        np.asarray(x, dtype=np.float32), np.asarray(cos, dtype=np.float32),
        np.asarray(sin, dtype=np.float32), np.asarray(w_qkv, dtype=np.float32),
        np.asarray(w_out, dtype=np.float32))

    res = bass_utils.run_bass_kernel_spmd(nc, in_maps, core_ids=list(range(8)))

    y = np.zeros((B, T, E), dtype=np.float32)
    for core in range(8):
        b = core // G
        y[b] += res.results[core]["y"].astype(np.float32)
    return y
